# revision 28
# baseline (speedup 1.0000x reference)
"""Bahdanau attention with coverage — Trainium2 Bass/Tile kernel.

Problem (per full input): B=32, T=4096, D=512
    enc = h_i @ W_h                         (B,T,D)
    dec = s_t @ W_s + b_s                   (B,D)
    cov = coverage[...,None] * W_c[0]       (B,T,D)
    e   = tanh(enc + dec[:,None,:] + cov) @ V[:,0]     (B,T)
    a   = softmax(e, axis=1)                (B,T)
    new_coverage = coverage + a
    context = einsum('bt,btd->bd', a, h_i)  (B,D)

Sharding: data-parallel over batch — each of the 8 cores gets 4 batches,
weights replicated.  No collectives needed.

Per-core kernel strategy (single pass over h_i):
  - h_i chunk [t=512, d=512] loaded naturally ([128p, 4, 512]); PE-transposes
    produce h^T (d on partitions) for the W_h matmul; GPSIMD casts the natural
    tile to bf16, kept resident for the context matmul.
  - features computed transposed [d_out, t]:  psum = sum_k W_h[k,:].T @ hT[k,:]
    + (K=1 matmul) W_c^T @ coverage-row.  dec folds in as the per-partition
    bias of the ACT tanh.
  - e^T chunks via M=1 matmuls with V stationary; col-tiling (tile_position)
    lands batch b's e row at psum partition 32*b, so all per-batch row math
    (exp, 1/Z, a_t, new_coverage, context scaling) is lane-local — no
    cross-partition moves anywhere.
  - softmax WITHOUT max subtraction (mathematically identical here: |e| <=
    ||V||_1 ~ 18, exp() cannot overflow fp32); Z comes free from ACT accum_out.
  - context accumulated on PE: lhsT = exp(e) column [128,1] (built by tiny PE
    transposes of the e row), rhs = resident natural bf16 h tile [128,512].
"""

import os

os.environ.setdefault("MYCRO_LOCAL_CACHE", "1")

from contextlib import ExitStack

import numpy as np

import concourse.bass as bass
import concourse.bacc as bacc
import concourse.mybir as mybir
import concourse.tile as tile
from concourse.bass_utils import run_bass_kernel_spmd
from concourse.masks import make_identity
from concourse.tile_rust import add_dep_helper

F32 = mybir.dt.float32
BF16 = mybir.dt.bfloat16
AF = mybir.ActivationFunctionType

B, T, D = 32, 4096, 512
NCORES = 8
BPC = B // NCORES            # batches per core = 4
TC = 512                     # t-chunk size
NTC = T // TC                # 8 chunks per batch
GROUP = 2                    # batches per resident group (SBUF budget)
P = 128


def _build_nc():
    nc = bacc.Bacc("TRN2")

    h_i = nc.dram_tensor("h_i", [BPC, T, D], F32, kind="ExternalInput")
    s_t = nc.dram_tensor("s_t", [BPC, D], F32, kind="ExternalInput")
    coverage = nc.dram_tensor("coverage", [BPC, T], F32, kind="ExternalInput")
    W_h = nc.dram_tensor("W_h", [D, D], F32, kind="ExternalInput")
    W_s = nc.dram_tensor("W_s", [D, D], F32, kind="ExternalInput")
    b_s = nc.dram_tensor("b_s", [D], F32, kind="ExternalInput")
    W_c = nc.dram_tensor("W_c", [1, D], F32, kind="ExternalInput")
    V = nc.dram_tensor("V", [D, 1], F32, kind="ExternalInput")

    ctx_out = nc.dram_tensor("context", [BPC, D], F32, kind="ExternalOutput")
    a_out = nc.dram_tensor("a_t", [BPC, T], F32, kind="ExternalOutput")
    ncov_out = nc.dram_tensor("new_coverage", [BPC, T], F32, kind="ExternalOutput")

    with tile.TileContext(nc) as tc, ExitStack() as ctx:
        _body(ctx, tc,
              h_i.ap(), s_t.ap(), coverage.ap(), W_h.ap(), W_s.ap(),
              b_s.ap(), W_c.ap(), V.ap(),
              ctx_out.ap(), a_out.ap(), ncov_out.ap())
    nc.finalize()
    return nc


def _body(ctx, tc, h_i, s_t, coverage, W_h, W_s, b_s, W_c, V,
          ctx_out, a_out, ncov_out):
    nc = tc.nc
    DC = D // P                      # 4 chunks of the feature dim

    singles = ctx.enter_context(tc.tile_pool(name="singles", bufs=1))
    stage = ctx.enter_context(tc.tile_pool(name="stage", bufs=1))
    hnat = ctx.enter_context(tc.tile_pool(name="hnat", bufs=3))
    hnatbf = ctx.enter_context(tc.tile_pool(name="hnatbf", bufs=GROUP * NTC + 3))
    htbf = ctx.enter_context(tc.tile_pool(name="htbf", bufs=2))
    tanhp = ctx.enter_context(tc.tile_pool(name="tanhp", bufs=2))
    smalls = ctx.enter_context(tc.tile_pool(name="smalls", bufs=2))

    ps_ht = ctx.enter_context(tc.tile_pool(name="ps_ht", bufs=2, space="PSUM"))
    ps_feat = ctx.enter_context(tc.tile_pool(name="ps_feat", bufs=2, space="PSUM"))
    ps_e = ctx.enter_context(tc.tile_pool(name="ps_e", bufs=1, space="PSUM"))
    ps_wc = ctx.enter_context(tc.tile_pool(name="ps_wc", bufs=1, space="PSUM"))
    ps_ctx = ctx.enter_context(tc.tile_pool(name="ps_ctx", bufs=2, space="PSUM"))

    # ---- constants & weight prep -------------------------------------------
    ident = singles.tile([P, P], F32)
    make_identity(nc, ident)
    ident_bf = singles.tile([P, P], BF16)
    nc.vector.tensor_copy(ident_bf, ident)
    ones4 = singles.tile([1, BPC], F32)
    nc.vector.memset(ones4, 1.0)

    # software-pipelined h loads: DMA + bf16 cast issued 2 chunks ahead
    NCHUNK = BPC * NTC
    pre = {}

    def issue_load(cidx):
        if cidx >= NCHUNK:
            return
        b2, t2 = divmod(cidx, NTC)
        h_nat = hnat.tile([P, 4, TC], F32, tag="h_nat")
        nc.sync.dma_start(
            out=h_nat,
            in_=h_i[b2, t2 * TC : (t2 + 1) * TC, :].rearrange(
                "(s p) d -> p s d", p=P),
        )
        h_bf = hnatbf.tile([P, 4, TC], BF16, tag="hbf")
        nc.gpsimd.tensor_copy(h_bf, h_nat)
        pre[cidx] = h_bf


    # small input rows first — they unblock the startup PE work (v/sT
    # transposes, dec matmuls) while the 1MiB h/W tiles stream in
    bs_sb = singles.tile([1, D], F32)
    nc.sync.dma_start(out=bs_sb, in_=b_s.rearrange("(o d) -> o d", o=1))
    wc_sb = singles.tile([1, D], F32)
    nc.sync.dma_start(out=wc_sb, in_=W_c)
    wc_bf = singles.tile([1, D], BF16)
    nc.vector.tensor_copy(wc_bf, wc_sb)
    v_row = singles.tile([1, D], F32)
    nc.sync.dma_start(out=v_row, in_=V.rearrange("d o -> o d"))
    st_sb = singles.tile([BPC, D], F32)
    nc.sync.dma_start(out=st_sb, in_=s_t)

    issue_load(0)
    issue_load(1)

    # W_h: [d_in, d_out] -> sbuf [p, c_in, d_out], cast to bf16
    wh_st = stage.tile([P, DC, D], F32, tag="wstage")
    nc.sync.dma_start(out=wh_st, in_=W_h.rearrange("(c p) e -> p c e", p=P))
    wh_bf = singles.tile([P, DC, D], BF16)
    nc.vector.tensor_copy(wh_bf, wh_st)

    # W_s stays fp32 (only used for tiny dec matmuls)
    ws_sb = stage.tile([P, DC, D], F32, tag="wstage2")
    nc.sync.dma_start(out=ws_sb, in_=W_s.rearrange("(c p) e -> p c e", p=P))

    # V: [D,1] -> row [1,D] -> PE-transpose into columns [128, DC] (bf16)
    v_ps = ps_wc.tile([P, DC], F32, tag="wcps")
    for c in range(DC):
        nc.tensor.transpose(v_ps[:, c : c + 1], v_row[0:1, c * P : (c + 1) * P],
                            ident[0:1, 0:1])
    v_bf = singles.tile([P, DC], BF16)
    nc.vector.tensor_copy(v_bf, v_ps)

    # s_t [BPC, D] -> s^T [p, c, b]
    sT_ps = ps_wc.tile([P, DC, BPC], F32, tag="wcps")
    for c in range(DC):
        nc.tensor.transpose(sT_ps[:, c, :], st_sb[0:BPC, c * P : (c + 1) * P],
                            ident[0:BPC, 0:BPC])
    sT_sb = singles.tile([P, DC, BPC], F32)
    nc.vector.tensor_copy(sT_sb, sT_ps)

    # dec^T[d_out, b] = W_s^T @ s^T + b_s  (per d_out chunk, N=BPC matmuls)
    dec_sb = singles.tile([P, DC, BPC], F32)
    for co in range(DC):
        dec_ps = ps_feat.tile([P, BPC], F32, tag="fps")
        for ci in range(DC):
            nc.tensor.matmul(dec_ps, ws_sb[:, ci, co * P : (co + 1) * P],
                             sT_sb[:, ci, :], start=(ci == 0), stop=False)
        nc.tensor.matmul(dec_ps, bs_sb[0:1, co * P : (co + 1) * P], ones4,
                         start=False, stop=True)
        nc.vector.tensor_copy(dec_sb[:, co, :], dec_ps)

    # per-batch rows live at partition 32*b — everything row-wise is lane-local
    e_sp = singles.tile([P, T], F32)      # e rows -> exp -> a_t (in place)
    cov_sp = singles.tile([P, T], F32)    # coverage rows -> new_coverage
    ctx_sp = singles.tile([P, D], F32)    # normalized contexts
    z_sp = singles.tile([P, 1], F32)
    rz_sp = singles.tile([P, 1], F32)
    for b in range(BPC):
        nc.sync.dma_start(out=cov_sp[32 * b : 32 * b + 1, :],
                          in_=coverage[b : b + 1, :])

    ngroups = BPC // GROUP
    pending_ctx = [None]

    def flush_ctx():
        if pending_ctx[0] is not None:
            fn, pending_ctx[0] = pending_ctx[0], None
            fn()

    for g in range(ngroups):
        h_keep = {}
        wc_ps = ps_wc.tile([P, GROUP, NTC * 4], F32, tag="wcps")
        for bg in range(GROUP):
            b = g * GROUP + bg
            bp = 32 * b

            # coverage row (bf16, partition 0) for the K=1 coverage matmul;
            # staged per chunk through a small fp32 tile
            cov_b = smalls.tile([1, T], BF16, tag="covb")

            for t in range(NTC):
                if t == 1:
                    flush_ctx()
                tsl = slice(t * TC, (t + 1) * TC)
                cov_st = smalls.tile([1, TC], F32, tag="covst")
                nc.sync.dma_start(out=cov_st, in_=coverage[b : b + 1, tsl])
                nc.vector.tensor_copy(cov_b[0:1, tsl], cov_st)

                # ---- prefetched natural tile (bf16) -----------------------
                cidx = b * NTC + t
                issue_load(cidx + 2)
                h_bf = pre.pop(cidx)
                h_keep[(bg, t)] = h_bf

                # ---- transpose: h^T [d_in, t] (bf16 in, 1 cyc/col) --------
                ht = htbf.tile([P, DC, TC], BF16)
                for ci in range(DC):
                    htp = ps_ht.tile([P, 4, P], BF16, tag="htps")
                    for j in range(4):
                        nc.tensor.transpose(
                            htp[:, j, :],
                            h_bf[:, j, ci * P : (ci + 1) * P],
                            ident_bf,
                        )
                    nc.vector.tensor_copy(ht[:, ci, :], htp)

                # ---- features^T + tanh ------------------------------------
                th = tanhp.tile([P, DC, TC], BF16)
                for co in range(DC):
                    fps = ps_feat.tile([P, TC], F32, tag="fps")
                    for ci in range(DC):
                        nc.tensor.matmul(
                            fps, wh_bf[:, ci, co * P : (co + 1) * P],
                            ht[:, ci, :], start=(ci == 0), stop=False)
                    nc.tensor.matmul(
                        fps, wc_bf[0:1, co * P : (co + 1) * P],
                        cov_b[0:1, tsl],
                        start=False, stop=True)
                    nc.scalar.activation(
                        out=th[:, co, :], in_=fps, func=AF.Tanh,
                        bias=dec_sb[:, co, b : b + 1], scale=1.0)

                # ---- e^T chunk -> psum partition 32*b ---------------------
                eps = ps_e.tile([P, TC], F32, tag="eps")
                for c in range(DC):
                    nc.tensor.matmul(eps[bp : bp + 1, :], v_bf[:, c : c + 1],
                                     th[:, c, :], start=(c == 0),
                                     stop=(c == DC - 1), tile_position=(0, bp))
                nc.vector.tensor_copy(e_sp[bp : bp + 1, tsl],
                                      eps[bp : bp + 1, :])

                # ---- e columns for the context matmul ---------------------
                for j in range(4):
                    seg = slice(t * TC + j * P, t * TC + (j + 1) * P)
                    nc.tensor.transpose(
                        wc_ps[:, bg, t * 4 + j : t * 4 + j + 1],
                        e_sp[bp : bp + 1, seg],
                        ident[bp : bp + 1, bp : bp + 1],
                        tile_position=(bp, 0),
                    )

        # ---- group end: exp of e-columns (straight from PSUM) -------------
        ew_bf = smalls.tile([P, GROUP, NTC * 4], BF16, tag="ewbf")
        ew_inst = nc.scalar.activation(out=ew_bf, in_=wc_ps, func=AF.Exp)

        # Row math for the whole group in single spanned ops: the group's
        # rows live at partitions 64g and 64g+32, so a 33-partition span
        # covers both batches per instruction (the 31 junk lanes in between
        # are computed but never read).
        gb = 32 * g * GROUP
        span = 32 * (GROUP - 1) + 1
        rexp = nc.scalar.activation(
            out=e_sp[gb : gb + span, :], in_=e_sp[gb : gb + span, :],
            func=AF.Exp, accum_out=z_sp[gb : gb + span, 0:1])
        add_dep_helper(rexp.ins, ew_inst.ins, sync=False,
                       reason="row exp after e-column exp")
        nc.vector.reciprocal(rz_sp[gb : gb + span, 0:1],
                             z_sp[gb : gb + span, 0:1])
        # a_t rows: exp * 1/Z in place; new_coverage in place
        nc.vector.tensor_scalar_mul(e_sp[gb : gb + span, :],
                                    e_sp[gb : gb + span, :],
                                    rz_sp[gb : gb + span, 0:1])
        nc.vector.tensor_add(cov_sp[gb : gb + span, :],
                             cov_sp[gb : gb + span, :],
                             e_sp[gb : gb + span, :])
        for bg in range(GROUP):
            b = g * GROUP + bg
            bp = 32 * b
            nc.sync.dma_start(out=a_out[b : b + 1, :], in_=e_sp[bp : bp + 1, :])
            nc.sync.dma_start(out=ncov_out[b : b + 1, :],
                              in_=cov_sp[bp : bp + 1, :])

        # context matmuls deferred: emitted after the next group's first
        # chunk so the PE never drains at the group boundary
        def make_ctx_phase(g, ew_bf, h_keep):
            def emit():
                cps_g = []
                for bg in range(GROUP):
                    cps = ps_ctx.tile([P, D], F32, tag="cps")
                    cps_g.append(cps)
                n = 0
                for t in range(NTC):
                    for j in range(4):
                        for bg in range(GROUP):
                            bp = 32 * (g * GROUP + bg)
                            nc.tensor.matmul(
                                cps_g[bg][bp : bp + 1, :],
                                ew_bf[:, bg, t * 4 + j : t * 4 + j + 1],
                                h_keep[(bg, t)][:, j, :],
                                start=(n == 0), stop=(n == NTC * 4 - 1),
                                tile_position=(0, bp))
                        n += 1
                for bg in range(GROUP):
                    b = g * GROUP + bg
                    bp = 32 * b
                    nc.vector.tensor_scalar_mul(ctx_sp[bp : bp + 1, :],
                                                cps_g[bg][bp : bp + 1, :],
                                                rz_sp[bp : bp + 1, 0:1])
                    nc.sync.dma_start(out=ctx_out[b : b + 1, :],
                                      in_=ctx_sp[bp : bp + 1, :])
            return emit

        pending_ctx[0] = make_ctx_phase(g, ew_bf, h_keep)
        if g == ngroups - 1:
            flush_ctx()


_NC_CACHE = None


def _get_nc():
    global _NC_CACHE
    if _NC_CACHE is None:
        _NC_CACHE = _build_nc()
    return _NC_CACHE


def kernel(h_i, s_t, coverage, W_h, W_s, b_s, W_c, V, _trace=False):
    h_i = np.ascontiguousarray(np.asarray(h_i, dtype=np.float32))
    s_t = np.ascontiguousarray(np.asarray(s_t, dtype=np.float32))
    coverage = np.ascontiguousarray(np.asarray(coverage, dtype=np.float32))
    W_h = np.ascontiguousarray(np.asarray(W_h, dtype=np.float32))
    W_s = np.ascontiguousarray(np.asarray(W_s, dtype=np.float32))
    b_s = np.ascontiguousarray(np.asarray(b_s, dtype=np.float32))
    W_c = np.ascontiguousarray(np.asarray(W_c, dtype=np.float32))
    V = np.ascontiguousarray(np.asarray(V, dtype=np.float32))

    nc = _get_nc()
    in_maps = []
    for c in range(NCORES):
        sl = slice(c * BPC, (c + 1) * BPC)
        in_maps.append({
            "h_i": h_i[sl], "s_t": s_t[sl], "coverage": coverage[sl],
            "W_h": W_h, "W_s": W_s, "b_s": b_s, "W_c": W_c, "V": V,
        })
    res = run_bass_kernel_spmd(nc, in_maps, list(range(NCORES)), trace=_trace)
    outs = res.results
    context = np.concatenate([r["context"] for r in outs], axis=0)
    a_t = np.concatenate([r["a_t"] for r in outs], axis=0)
    new_coverage = np.concatenate([r["new_coverage"] for r in outs], axis=0)
    if _trace:
        kernel.last_exec_time_ns = res.exec_time_ns
        kernel.last_results = res
    return context, a_t, new_coverage


# revision 29
# speedup vs baseline: 1.0011x; 1.0011x over previous
"""Bahdanau attention with coverage — Trainium2 Bass/Tile kernel.

Problem (per full input): B=32, T=4096, D=512
    enc = h_i @ W_h                         (B,T,D)
    dec = s_t @ W_s + b_s                   (B,D)
    cov = coverage[...,None] * W_c[0]       (B,T,D)
    e   = tanh(enc + dec[:,None,:] + cov) @ V[:,0]     (B,T)
    a   = softmax(e, axis=1)                (B,T)
    new_coverage = coverage + a
    context = einsum('bt,btd->bd', a, h_i)  (B,D)

Sharding: data-parallel over batch — each of the 8 cores gets 4 batches,
weights replicated.  No collectives needed.

Per-core kernel strategy (single pass over h_i):
  - h_i chunk [t=512, d=512] loaded naturally ([128p, 4, 512]); PE-transposes
    produce h^T (d on partitions) for the W_h matmul; GPSIMD casts the natural
    tile to bf16, kept resident for the context matmul.
  - features computed transposed [d_out, t]:  psum = sum_k W_h[k,:].T @ hT[k,:]
    + (K=1 matmul) W_c^T @ coverage-row.  dec folds in as the per-partition
    bias of the ACT tanh.
  - e^T chunks via M=1 matmuls with V stationary; col-tiling (tile_position)
    lands batch b's e row at psum partition 32*b, so all per-batch row math
    (exp, 1/Z, a_t, new_coverage, context scaling) is lane-local — no
    cross-partition moves anywhere.
  - softmax WITHOUT max subtraction (mathematically identical here: |e| <=
    ||V||_1 ~ 18, exp() cannot overflow fp32); Z comes free from ACT accum_out.
  - context accumulated on PE: lhsT = exp(e) column [128,1] (built by tiny PE
    transposes of the e row), rhs = resident natural bf16 h tile [128,512].
"""

import os

os.environ.setdefault("MYCRO_LOCAL_CACHE", "1")

from contextlib import ExitStack

import numpy as np

import concourse.bass as bass
import concourse.bacc as bacc
import concourse.mybir as mybir
import concourse.tile as tile
from concourse.bass_utils import run_bass_kernel_spmd
from concourse.masks import make_identity
from concourse.tile_rust import add_dep_helper

F32 = mybir.dt.float32
BF16 = mybir.dt.bfloat16
AF = mybir.ActivationFunctionType

B, T, D = 32, 4096, 512
NCORES = 8
BPC = B // NCORES            # batches per core = 4
TC = 512                     # t-chunk size
NTC = T // TC                # 8 chunks per batch
GROUP = 2                    # batches per resident group (SBUF budget)
P = 128


def _build_nc():
    nc = bacc.Bacc("TRN2")

    h_i = nc.dram_tensor("h_i", [BPC, T, D], F32, kind="ExternalInput")
    s_t = nc.dram_tensor("s_t", [BPC, D], F32, kind="ExternalInput")
    coverage = nc.dram_tensor("coverage", [BPC, T], F32, kind="ExternalInput")
    W_h = nc.dram_tensor("W_h", [D, D], F32, kind="ExternalInput")
    W_s = nc.dram_tensor("W_s", [D, D], F32, kind="ExternalInput")
    b_s = nc.dram_tensor("b_s", [D], F32, kind="ExternalInput")
    W_c = nc.dram_tensor("W_c", [1, D], F32, kind="ExternalInput")
    V = nc.dram_tensor("V", [D, 1], F32, kind="ExternalInput")

    ctx_out = nc.dram_tensor("context", [BPC, D], F32, kind="ExternalOutput")
    a_out = nc.dram_tensor("a_t", [BPC, T], F32, kind="ExternalOutput")
    ncov_out = nc.dram_tensor("new_coverage", [BPC, T], F32, kind="ExternalOutput")

    with tile.TileContext(nc) as tc, ExitStack() as ctx:
        _body(ctx, tc,
              h_i.ap(), s_t.ap(), coverage.ap(), W_h.ap(), W_s.ap(),
              b_s.ap(), W_c.ap(), V.ap(),
              ctx_out.ap(), a_out.ap(), ncov_out.ap())
    nc.finalize()
    return nc


def _body(ctx, tc, h_i, s_t, coverage, W_h, W_s, b_s, W_c, V,
          ctx_out, a_out, ncov_out):
    nc = tc.nc
    DC = D // P                      # 4 chunks of the feature dim

    singles = ctx.enter_context(tc.tile_pool(name="singles", bufs=1))
    stage = ctx.enter_context(tc.tile_pool(name="stage", bufs=1))
    hnat = ctx.enter_context(tc.tile_pool(name="hnat", bufs=3))
    hnatbf = ctx.enter_context(tc.tile_pool(name="hnatbf", bufs=GROUP * NTC + 3))
    htbf = ctx.enter_context(tc.tile_pool(name="htbf", bufs=2))
    tanhp = ctx.enter_context(tc.tile_pool(name="tanhp", bufs=3))
    smalls = ctx.enter_context(tc.tile_pool(name="smalls", bufs=2))

    ps_ht = ctx.enter_context(tc.tile_pool(name="ps_ht", bufs=2, space="PSUM"))
    ps_feat = ctx.enter_context(tc.tile_pool(name="ps_feat", bufs=2, space="PSUM"))
    ps_e = ctx.enter_context(tc.tile_pool(name="ps_e", bufs=1, space="PSUM"))
    ps_wc = ctx.enter_context(tc.tile_pool(name="ps_wc", bufs=1, space="PSUM"))
    ps_ctx = ctx.enter_context(tc.tile_pool(name="ps_ctx", bufs=2, space="PSUM"))

    # ---- constants & weight prep -------------------------------------------
    ident = singles.tile([P, P], F32)
    make_identity(nc, ident)
    ident_bf = singles.tile([P, P], BF16)
    nc.vector.tensor_copy(ident_bf, ident)
    ones4 = singles.tile([1, BPC], F32)
    nc.vector.memset(ones4, 1.0)

    # software-pipelined h loads: DMA + bf16 cast issued 2 chunks ahead.
    # Chunk order interleaves the group's batches: (b0,t0),(b1,t0),(b0,t1)...
    ngroups_ = BPC // GROUP
    ORDER = [(gg * GROUP + bgg, tt)
             for gg in range(ngroups_)
             for tt in range(NTC)
             for bgg in range(GROUP)]
    pre = {}

    def issue_load(seq):
        if seq >= len(ORDER):
            return
        b2, t2 = ORDER[seq]
        h_nat = hnat.tile([P, 4, TC], F32, tag="h_nat")
        nc.sync.dma_start(
            out=h_nat,
            in_=h_i[b2, t2 * TC : (t2 + 1) * TC, :].rearrange(
                "(s p) d -> p s d", p=P),
        )
        h_bf = hnatbf.tile([P, 4, TC], BF16, tag="hbf")
        nc.gpsimd.tensor_copy(h_bf, h_nat)
        pre[(b2, t2)] = h_bf


    # small input rows first — they unblock the startup PE work (v/sT
    # transposes, dec matmuls) while the 1MiB h/W tiles stream in
    bs_sb = singles.tile([1, D], F32)
    nc.sync.dma_start(out=bs_sb, in_=b_s.rearrange("(o d) -> o d", o=1))
    wc_sb = singles.tile([1, D], F32)
    nc.sync.dma_start(out=wc_sb, in_=W_c)
    wc_bf = singles.tile([1, D], BF16)
    nc.vector.tensor_copy(wc_bf, wc_sb)
    v_row = singles.tile([1, D], F32)
    nc.sync.dma_start(out=v_row, in_=V.rearrange("d o -> o d"))
    st_sb = singles.tile([BPC, D], F32)
    nc.sync.dma_start(out=st_sb, in_=s_t)

    issue_load(0)
    issue_load(1)

    # W_h: [d_in, d_out] -> sbuf [p, c_in, d_out], cast to bf16
    wh_st = stage.tile([P, DC, D], F32, tag="wstage")
    nc.sync.dma_start(out=wh_st, in_=W_h.rearrange("(c p) e -> p c e", p=P))
    wh_bf = singles.tile([P, DC, D], BF16)
    nc.vector.tensor_copy(wh_bf, wh_st)

    # W_s stays fp32 (only used for tiny dec matmuls)
    ws_sb = stage.tile([P, DC, D], F32, tag="wstage2")
    nc.sync.dma_start(out=ws_sb, in_=W_s.rearrange("(c p) e -> p c e", p=P))

    # V: [D,1] -> row [1,D] -> PE-transpose into columns [128, DC] (bf16)
    v_ps = ps_wc.tile([P, DC], F32, tag="wcps")
    for c in range(DC):
        nc.tensor.transpose(v_ps[:, c : c + 1], v_row[0:1, c * P : (c + 1) * P],
                            ident[0:1, 0:1])
    v_bf = singles.tile([P, DC], BF16)
    nc.vector.tensor_copy(v_bf, v_ps)

    # s_t [BPC, D] -> s^T [p, c, b]
    sT_ps = ps_wc.tile([P, DC, BPC], F32, tag="wcps")
    for c in range(DC):
        nc.tensor.transpose(sT_ps[:, c, :], st_sb[0:BPC, c * P : (c + 1) * P],
                            ident[0:BPC, 0:BPC])
    sT_sb = singles.tile([P, DC, BPC], F32)
    nc.vector.tensor_copy(sT_sb, sT_ps)

    # dec^T[d_out, b] = W_s^T @ s^T + b_s  (per d_out chunk, N=BPC matmuls)
    dec_sb = singles.tile([P, DC, BPC], F32)
    for co in range(DC):
        dec_ps = ps_feat.tile([P, BPC], F32, tag="fps")
        for ci in range(DC):
            nc.tensor.matmul(dec_ps, ws_sb[:, ci, co * P : (co + 1) * P],
                             sT_sb[:, ci, :], start=(ci == 0), stop=False)
        nc.tensor.matmul(dec_ps, bs_sb[0:1, co * P : (co + 1) * P], ones4,
                         start=False, stop=True)
        nc.vector.tensor_copy(dec_sb[:, co, :], dec_ps)

    # per-batch rows live at partition 32*b — everything row-wise is lane-local
    e_sp = singles.tile([P, T], F32)      # e rows -> exp -> a_t (in place)
    cov_sp = singles.tile([P, T], F32)    # coverage rows -> new_coverage
    ctx_sp = singles.tile([P, D], F32)    # normalized contexts
    z_sp = singles.tile([P, 1], F32)
    rz_sp = singles.tile([P, 1], F32)
    for b in range(BPC):
        nc.sync.dma_start(out=cov_sp[32 * b : 32 * b + 1, :],
                          in_=coverage[b : b + 1, :])

    ngroups = BPC // GROUP
    pending_ctx = [None]

    def flush_ctx():
        if pending_ctx[0] is not None:
            fn, pending_ctx[0] = pending_ctx[0], None
            fn()

    for g in range(ngroups):
        h_keep = {}
        wc_ps = ps_wc.tile([P, GROUP, NTC * 4], F32, tag="wcps")
        cov_bs = []
        for bg in range(GROUP):
            cov_b = smalls.tile([1, T], BF16, tag="covb")
            cov_bs.append(cov_b)

        for t in range(NTC):
            if t == 1:
                flush_ctx()
            tsl = slice(t * TC, (t + 1) * TC)
            # shared e psum bank: batch bg writes partition 32*b
            eps = ps_e.tile([P, TC], F32, tag="eps")
            for bg in range(GROUP):
                b = g * GROUP + bg
                bp = 32 * b
                cov_b = cov_bs[bg]
                cov_st = smalls.tile([1, TC], F32, tag="covst")
                nc.sync.dma_start(out=cov_st, in_=coverage[b : b + 1, tsl])
                nc.vector.tensor_copy(cov_b[0:1, tsl], cov_st)

                # ---- prefetched natural tile (bf16) -----------------------
                seq = (g * NTC + t) * GROUP + bg
                issue_load(seq + 2)
                h_bf = pre.pop((b, t))
                h_keep[(bg, t)] = h_bf

                # ---- transpose: h^T [d_in, t] (bf16 in, 1 cyc/col) --------
                ht = htbf.tile([P, DC, TC], BF16)
                for ci in range(DC):
                    htp = ps_ht.tile([P, 4, P], BF16, tag="htps")
                    for j in range(4):
                        nc.tensor.transpose(
                            htp[:, j, :],
                            h_bf[:, j, ci * P : (ci + 1) * P],
                            ident_bf,
                        )
                    nc.vector.tensor_copy(ht[:, ci, :], htp)

                # ---- features^T + tanh ------------------------------------
                th = tanhp.tile([P, DC, TC], BF16)
                for co in range(DC):
                    fps = ps_feat.tile([P, TC], F32, tag="fps")
                    for ci in range(DC):
                        nc.tensor.matmul(
                            fps, wh_bf[:, ci, co * P : (co + 1) * P],
                            ht[:, ci, :], start=(ci == 0), stop=False)
                    nc.tensor.matmul(
                        fps, wc_bf[0:1, co * P : (co + 1) * P],
                        cov_b[0:1, tsl],
                        start=False, stop=True)
                    nc.scalar.activation(
                        out=th[:, co, :], in_=fps, func=AF.Tanh,
                        bias=dec_sb[:, co, b : b + 1], scale=1.0)

                # ---- e^T chunk -> psum partition 32*b ---------------------
                for c in range(DC):
                    nc.tensor.matmul(eps[bp : bp + 1, :], v_bf[:, c : c + 1],
                                     th[:, c, :], start=(c == 0),
                                     stop=(c == DC - 1), tile_position=(0, bp))
                nc.vector.tensor_copy(e_sp[bp : bp + 1, tsl],
                                      eps[bp : bp + 1, :])

                # ---- e columns for the context matmul ---------------------
                for j in range(4):
                    seg = slice(t * TC + j * P, t * TC + (j + 1) * P)
                    nc.tensor.transpose(
                        wc_ps[:, bg, t * 4 + j : t * 4 + j + 1],
                        e_sp[bp : bp + 1, seg],
                        ident[bp : bp + 1, bp : bp + 1],
                        tile_position=(bp, 0),
                    )

        # ---- group end: exp of e-columns (straight from PSUM) -------------
        ew_bf = smalls.tile([P, GROUP, NTC * 4], BF16, tag="ewbf")
        ew_inst = nc.scalar.activation(out=ew_bf, in_=wc_ps, func=AF.Exp)

        # Row math for the whole group in single spanned ops: the group's
        # rows live at partitions 64g and 64g+32, so a 33-partition span
        # covers both batches per instruction (the 31 junk lanes in between
        # are computed but never read).
        gb = 32 * g * GROUP
        span = 32 * (GROUP - 1) + 1
        rexp = nc.scalar.activation(
            out=e_sp[gb : gb + span, :], in_=e_sp[gb : gb + span, :],
            func=AF.Exp, accum_out=z_sp[gb : gb + span, 0:1])
        add_dep_helper(rexp.ins, ew_inst.ins, sync=False,
                       reason="row exp after e-column exp")
        nc.vector.reciprocal(rz_sp[gb : gb + span, 0:1],
                             z_sp[gb : gb + span, 0:1])
        # a_t rows: exp * 1/Z in place; new_coverage in place
        nc.vector.tensor_scalar_mul(e_sp[gb : gb + span, :],
                                    e_sp[gb : gb + span, :],
                                    rz_sp[gb : gb + span, 0:1])
        nc.vector.tensor_add(cov_sp[gb : gb + span, :],
                             cov_sp[gb : gb + span, :],
                             e_sp[gb : gb + span, :])
        for bg in range(GROUP):
            b = g * GROUP + bg
            bp = 32 * b
            nc.sync.dma_start(out=a_out[b : b + 1, :], in_=e_sp[bp : bp + 1, :])
            nc.sync.dma_start(out=ncov_out[b : b + 1, :],
                              in_=cov_sp[bp : bp + 1, :])

        # context matmuls deferred: emitted after the next group's first
        # chunk so the PE never drains at the group boundary
        def make_ctx_phase(g, ew_bf, h_keep):
            def emit():
                cps_g = []
                for bg in range(GROUP):
                    cps = ps_ctx.tile([P, D], F32, tag="cps")
                    cps_g.append(cps)
                n = 0
                for t in range(NTC):
                    for j in range(4):
                        for bg in range(GROUP):
                            bp = 32 * (g * GROUP + bg)
                            nc.tensor.matmul(
                                cps_g[bg][bp : bp + 1, :],
                                ew_bf[:, bg, t * 4 + j : t * 4 + j + 1],
                                h_keep[(bg, t)][:, j, :],
                                start=(n == 0), stop=(n == NTC * 4 - 1),
                                tile_position=(0, bp))
                        n += 1
                for bg in range(GROUP):
                    b = g * GROUP + bg
                    bp = 32 * b
                    nc.vector.tensor_scalar_mul(ctx_sp[bp : bp + 1, :],
                                                cps_g[bg][bp : bp + 1, :],
                                                rz_sp[bp : bp + 1, 0:1])
                    nc.sync.dma_start(out=ctx_out[b : b + 1, :],
                                      in_=ctx_sp[bp : bp + 1, :])
            return emit

        pending_ctx[0] = make_ctx_phase(g, ew_bf, h_keep)
        if g == ngroups - 1:
            flush_ctx()


_NC_CACHE = None


def _get_nc():
    global _NC_CACHE
    if _NC_CACHE is None:
        _NC_CACHE = _build_nc()
    return _NC_CACHE


def kernel(h_i, s_t, coverage, W_h, W_s, b_s, W_c, V, _trace=False):
    h_i = np.ascontiguousarray(np.asarray(h_i, dtype=np.float32))
    s_t = np.ascontiguousarray(np.asarray(s_t, dtype=np.float32))
    coverage = np.ascontiguousarray(np.asarray(coverage, dtype=np.float32))
    W_h = np.ascontiguousarray(np.asarray(W_h, dtype=np.float32))
    W_s = np.ascontiguousarray(np.asarray(W_s, dtype=np.float32))
    b_s = np.ascontiguousarray(np.asarray(b_s, dtype=np.float32))
    W_c = np.ascontiguousarray(np.asarray(W_c, dtype=np.float32))
    V = np.ascontiguousarray(np.asarray(V, dtype=np.float32))

    nc = _get_nc()
    in_maps = []
    for c in range(NCORES):
        sl = slice(c * BPC, (c + 1) * BPC)
        in_maps.append({
            "h_i": h_i[sl], "s_t": s_t[sl], "coverage": coverage[sl],
            "W_h": W_h, "W_s": W_s, "b_s": b_s, "W_c": W_c, "V": V,
        })
    res = run_bass_kernel_spmd(nc, in_maps, list(range(NCORES)), trace=_trace)
    outs = res.results
    context = np.concatenate([r["context"] for r in outs], axis=0)
    a_t = np.concatenate([r["a_t"] for r in outs], axis=0)
    new_coverage = np.concatenate([r["new_coverage"] for r in outs], axis=0)
    if _trace:
        kernel.last_exec_time_ns = res.exec_time_ns
        kernel.last_results = res
    return context, a_t, new_coverage
